# revision 18
# baseline (speedup 1.0000x reference)
"""Trainium2 Bass kernel for nn_GCGC_32006096290301 (gnn_message_passing).

Data-parallel over batch B=64 across 8 NeuronCores (8 batches/core).
Per batch b on-chip pipeline:
  xa[c,(t,v)] -> xmean -> x1m/x2m (one PE matmul) -> rel=tanh(x1-x2) -> ridge
  -> banded weight tensor WBs[(dt,v),(slot,i)]
  xa --PE transpose--> XW2[(dt,v),(c,u)]  (t',v on partitions, 5-aligned t' windows)
  conv: z^[c,(trel,i)] accumulated in PSUM from window matmuls
        (contraction over (dt,v)=125, k-taps folded into the band slots)
  z -> zsb; loss partial = sum((z-x)^2); zo = w3 @ z + b3 -> DRAM
Returns (zo[B,O,T,V], loss scalar, ridge[B]) like the reference.
"""
import os
import sys

sys.path.insert(0, '/opt/trn_rl_repo')

import numpy as np
import concourse.bass as bass
import concourse.bacc as bacc
import concourse.tile as tile
from concourse import mybir
from concourse.bass_utils import run_bass_kernel_spmd
from concourse.masks import make_identity

B, C, T, V, O, S = 64, 64, 256, 25, 256, 9
NCORES = 8
PB = B // NCORES
NT = 20           # output t's per conv psum tile (one 512-f32 PSUM bank = 500 cols)
KV = 5 * V        # 125 contraction partitions (5 t' x 25 v)

# dtype modes: 'f32' | 'f32r' | 'bf16'
CONV_MODE = os.environ.get("K_CONV", "f32")
MM3_MODE = os.environ.get("K_MM3", "f32")
BAND = os.environ.get("K_BAND", "wide")   # 'wide' (sim-safe) | 'tight' (HW per-element psum semantics)

F32 = mybir.dt.float32
F32R = mybir.dt.float32r
BF16 = mybir.dt.bfloat16
AF = mybir.ActivationFunctionType


def _ap(t, offset, dims):
    return bass.AP(tensor=t.tensor if hasattr(t, 'tensor') else t, offset=offset, ap=dims)


def build_program(pb=PB, c=C, t=T, o=O, conv_mode=CONV_MODE, mm3_mode=MM3_MODE, band=BAND):
    tv = t * V
    ntiles = (t + NT - 1) // NT
    u_lo, u_hi = 1, (t + 7) // 5            # valid transpose windows (inclusive)
    n_u = 4 * (ntiles - 1) + 6              # u slots 0 .. 4*(ntiles-1)+5
    o_chunks = [(i, min(128, o - i)) for i in range(0, o, 128)]
    conv_dt = {'f32': F32, 'f32r': F32, 'bf16': BF16}[conv_mode]
    wide = band == 'wide'
    n_slot = 45 if wide else 13             # band slots: wide r in [-25,20), tight r in [-8,5)

    nc = bacc.Bacc("TRN2", target_bir_lowering=False, debug=False,
                   enable_asserts=False, num_devices=NCORES)

    x_d = nc.dram_tensor("x", [pb, c, tv], F32, kind="ExternalInput").ap()
    w1_d = nc.dram_tensor("w1", [S, c], F32, kind="ExternalInput").ap()
    b1_d = nc.dram_tensor("b1", [S], F32, kind="ExternalInput").ap()
    w2_d = nc.dram_tensor("w2", [S, c], F32, kind="ExternalInput").ap()
    b2_d = nc.dram_tensor("b2", [S], F32, kind="ExternalInput").ap()
    w3_d = nc.dram_tensor("w3", [o, c], F32, kind="ExternalInput").ap()
    b3_d = nc.dram_tensor("b3", [o], F32, kind="ExternalInput").ap()
    A_d = nc.dram_tensor("A", [V, V], F32, kind="ExternalInput").ap()
    zo_d = nc.dram_tensor("zo", [pb, o, tv], F32, kind="ExternalOutput").ap()
    st_d = nc.dram_tensor("stats", [pb, 2], F32, kind="ExternalOutput").ap()

    from contextlib import ExitStack
    with tile.TileContext(nc) as tc, ExitStack() as ctx:
        consts = ctx.enter_context(tc.tile_pool(name="consts", bufs=1))
        xap = ctx.enter_context(tc.tile_pool(name="xa", bufs=2))
        xwp = ctx.enter_context(tc.tile_pool(name="xw", bufs=2))
        zsp = ctx.enter_context(tc.tile_pool(name="zs", bufs=2))
        wbp = ctx.enter_context(tc.tile_pool(name="wb", bufs=2))
        smp = ctx.enter_context(tc.tile_pool(name="sm", bufs=3))
        zop = ctx.enter_context(tc.tile_pool(name="zo", bufs=4))
        drp = ctx.enter_context(tc.tile_pool(name="dr", bufs=2, space="DRAM"))
        tp_ps = ctx.enter_context(tc.tile_pool(name="tp_ps", bufs=2, space="PSUM"))
        cv_ps = ctx.enter_context(tc.tile_pool(name="cv_ps", bufs=2, space="PSUM"))
        m3_ps = ctx.enter_context(tc.tile_pool(name="m3_ps", bufs=2, space="PSUM"))
        sm_ps = ctx.enter_context(tc.tile_pool(name="sm_ps", bufs=2, space="PSUM"))

        # ---------------- constants / weight preprocessing ----------------
        ident = consts.tile([128, 128], F32)
        make_identity(nc, ident)
        ones = consts.tile([128, 1], F32)
        nc.vector.memset(ones, 1.0)

        # w12T [c, 18]: w12T[cc, s] = w1[s, cc], cols 9..17 from w2
        w12T = consts.tile([c, 2 * S], F32)
        nc.sync.dma_start(out=w12T[:, 0:S], in_=_ap(w1_d, 0, [[1, c], [c, S]]))
        nc.sync.dma_start(out=w12T[:, S:2 * S], in_=_ap(w2_d, 0, [[1, c], [c, S]]))
        b12rep = consts.tile([V, 2 * S], F32)
        nc.sync.dma_start(out=b12rep[:, 0:S], in_=_ap(b1_d, 0, [[0, V], [1, S]]))
        nc.sync.dma_start(out=b12rep[:, S:2 * S], in_=_ap(b2_d, 0, [[0, V], [1, S]]))
        # A_rep [V, S, V]: A broadcast over k
        A_rep = consts.tile([V, S, V], F32)
        nc.sync.dma_start(out=A_rep[:], in_=_ap(A_d, 0, [[V, V], [0, S], [1, V]]))
        # b3c [128, n_och]
        b3c = consts.tile([128, len(o_chunks)], F32)
        for i, (o0, osz) in enumerate(o_chunks):
            nc.sync.dma_start(out=b3c[0:osz, i:i + 1], in_=_ap(b3_d, o0, [[1, osz], [0, 1]]))
        # w3T [c, o] via PE transposes of w3 [o, c]
        w3T = consts.tile([c, o], F32)
        for o0, osz in o_chunks:
            w3n = smp.tile([128, c], F32, tag="w3n")
            nc.sync.dma_start(out=w3n[0:osz, :], in_=w3_d[o0:o0 + osz, :])
            ps = tp_ps.tile([c, 128], F32, tag="tp")
            nc.tensor.transpose(ps[:, 0:osz], w3n[0:osz, :], ident[0:osz, 0:osz])
            nc.vector.tensor_copy(w3T[:, o0:o0 + osz], ps[:, 0:osz])
        if mm3_mode == 'bf16':
            w3Tb = consts.tile([c, o], BF16)
            nc.scalar.activation(out=w3Tb[:], in_=w3T[:], func=AF.Copy)

        lossB = consts.tile([c, pb], F32)
        ridgeB = consts.tile([V, pb], F32)

        # ---------------- per-batch pipeline ----------------
        # xa_pad holds 8 zero t-slots in front and 4 after: padded t'' = t' + 8.
        # All transpose windows u cover t'' in [5u, 5u+5) fully -> no edge cases.
        PADF, PADB = 8, 4
        tp_total = PADF + t + PADB
        x_off = PADF * V
        for b in range(pb):
            xa = xap.tile([c, tp_total * V], F32)
            nc.gpsimd.memset(xa[:, 0:x_off], 0.0)
            nc.gpsimd.memset(xa[:, x_off + tv:tp_total * V], 0.0)
            nc.sync.dma_start(out=xa[:, x_off:x_off + tv], in_=x_d[b])

            # xmean[c, v] = sum_t xa / t
            xmean = smp.tile([c, V], F32)
            nc.vector.reduce_sum(out=xmean[:],
                                 in_=xa[:, x_off:x_off + tv].rearrange("p (t v) -> p v t", v=V),
                                 axis=mybir.AxisListType.X)
            xmean_s = smp.tile([c, V], F32)
            nc.scalar.activation(out=xmean_s[:], in_=xmean[:], func=AF.Copy, scale=1.0 / t)

            # x12T [V, 2S] = xmean_s.T @ w12T  (+ b12rep)
            ps12 = sm_ps.tile([V, 2 * S], F32, tag="sm")
            nc.tensor.matmul(ps12[:], xmean_s[:], w12T[:], start=True, stop=True)
            x12 = smp.tile([V, 2 * S], F32)
            nc.vector.tensor_add(x12[:], ps12[:], b12rep[:])

            # x2 flatten via DRAM scratch -> x2rep [V, S, V]
            scr = drp.tile([1, S * V], F32)
            nc.sync.dma_start(out=_ap(scr, 0, [[1, V], [V, S]]), in_=x12[:, S:2 * S])
            x2rep = smp.tile([V, S, V], F32)
            nc.sync.dma_start(out=x2rep[:], in_=_ap(scr, 0, [[0, V], [V, S], [1, V]]))

            # rel[v, kd, i] = tanh(x1[k,v] - x2[k,i]),  kd = S-1-k
            rel = smp.tile([V, S, V], F32)
            for k in range(S):
                kd = S - 1 - k
                nc.vector.tensor_scalar(out=rel[:, kd, :], in0=x2rep[:, k, :],
                                        scalar1=x12[:, k:k + 1], scalar2=None,
                                        op0=mybir.AluOpType.subtract)
            nc.scalar.activation(out=rel[:], in_=rel[:], func=AF.Tanh, scale=-1.0)
            # ridge partial
            rsq = smp.tile([V, S, V], F32)
            nc.scalar.activation(out=rsq[:], in_=rel[:], func=AF.Square,
                                 accum_out=ridgeB[:, b:b + 1])
            # Wrev = rel + A  (columns already k-descending)
            wrev = smp.tile([V, S, V], F32)
            nc.vector.tensor_add(wrev[:], rel[:], A_rep[:])

            # WBs [(dt,v)=KV, n_slot, V]: 5 shifted copies of wrev (only DMA can
            # write at 25-aligned partition offsets). wbs_raw -> wbs via one DVE
            # copy so conv matmuls depend on a single producer (walrus limits
            # sync-wait commands per LDW/matmul).
            wbs_raw = wbp.tile([KV, n_slot, V], conv_dt, tag="wbs_raw")
            nc.gpsimd.memset(wbs_raw[:], 0.0)
            base = 17 if wide else 0
            if conv_dt == BF16:
                wrev_c = smp.tile([V, S, V], BF16, tag="wrevb")
                nc.scalar.activation(out=wrev_c[:], in_=wrev[:], func=AF.Copy)
            else:
                wrev_c = wrev
            for dt_ in range(5):
                nc.sync.dma_start(out=wbs_raw[dt_ * V:(dt_ + 1) * V, base + dt_:base + dt_ + S, :],
                                  in_=wrev_c[:])
            wbs = wbp.tile([KV, n_slot, V], conv_dt, tag="wbs")
            nc.vector.tensor_copy(wbs[:], wbs_raw[:])

            # XW2 [(dt,v)=KV, c, n_u]: XW2[dt*V+v, cc, u] = x[cc, t'=5u-8+dt, v]
            xw2 = xwp.tile([KV, c, n_u], conv_dt)
            for u in range(0, min(u_hi, n_u - 1) + 1):
                # window u covers padded t'' in [5u, 5u+5) -> always full
                tps = tp_ps.tile([KV, c], F32, tag="tp")
                nc.tensor.transpose(tps[:], xa[:, 5 * u * V:(5 * u + 5) * V], ident[0:c, 0:c])
                nc.vector.tensor_copy(xw2[:, :, u], tps[:])

            # conv: psum tiles over t
            zsb = zsp.tile([c, tv], F32)
            if mm3_mode == 'bf16':
                zsbb = zsp.tile([c, tv], BF16, tag="zsbb")
            lossT = smp.tile([c, ntiles], F32, tag="lossT")
            for m in range(ntiles):
                nt_eff = min(NT, t - NT * m)
                cps = cv_ps.tile([c, NT * V], F32, tag="conv")
                jmax = min(5, (nt_eff + 7) // 5)
                js = []
                for j in range(jmax + 1):
                    u = 4 * m + j
                    t0p = 5 * u - 8
                    if t0p + 5 <= 0 or t0p >= t:
                        continue
                    js.append(j)
                for idx, j in enumerate(js):
                    u = 4 * m + j
                    lhsT = xw2[:, :, u]
                    if wide:
                        rhs = wbs[:, 25 - 5 * j:25 - 5 * j + nt_eff, :]
                        out_ap = cps[:, 0:nt_eff * V]
                    else:
                        lo = max(0, 5 * j - 8)
                        hi = min(nt_eff, 5 * j + 5)
                        rhs = wbs[:, lo - 5 * j + 8:hi - 5 * j + 8, :]
                        out_ap = cps[:, lo * V:hi * V]
                    if conv_mode == 'f32r':
                        lhsT = lhsT.bitcast(F32R)
                        rhs = rhs.bitcast(F32R)
                    nc.tensor.matmul(out_ap, lhsT, rhs,
                                     start=(idx == 0), stop=(idx == len(js) - 1))
                nc.scalar.activation(out=zsb[:, m * NT * V:m * NT * V + nt_eff * V],
                                     in_=cps[:, 0:nt_eff * V], func=AF.Copy)
                if mm3_mode == 'bf16':
                    nc.vector.tensor_copy(zsbb[:, m * NT * V:m * NT * V + nt_eff * V],
                                          cps[:, 0:nt_eff * V])
                # loss partial for this tile: sum((z - x)^2) -> lossT[:, m]
                dtile = smp.tile([c, NT * V], F32, tag="dtile")
                nc.vector.tensor_sub(dtile[:, 0:nt_eff * V], cps[:, 0:nt_eff * V],
                                     xa[:, x_off + m * NT * V:x_off + m * NT * V + nt_eff * V])
                nc.scalar.activation(out=dtile[:, 0:nt_eff * V], in_=dtile[:, 0:nt_eff * V],
                                     func=AF.Square, accum_out=lossT[:, m:m + 1])

            # per-batch loss partial
            nc.vector.reduce_sum(out=lossB[:, b:b + 1], in_=lossT[:], axis=mybir.AxisListType.X)

            # conv3: zo = w3 @ z + b3
            ncol = 512
            for oi, (o0, osz) in enumerate(o_chunks):
                for n0 in range(0, tv, ncol):
                    nsz = min(ncol, tv - n0)
                    mps = m3_ps.tile([128, ncol], F32, tag="mm3")
                    if mm3_mode == 'bf16':
                        lhsT = w3Tb[:, o0:o0 + osz]
                        rhs = zsbb[:, n0:n0 + nsz]
                    elif mm3_mode == 'f32r':
                        lhsT = w3T[:, o0:o0 + osz].bitcast(F32R)
                        rhs = zsb[:, n0:n0 + nsz].bitcast(F32R)
                    else:
                        lhsT = w3T[:, o0:o0 + osz]
                        rhs = zsb[:, n0:n0 + nsz]
                    nc.tensor.matmul(mps[0:osz, 0:nsz], lhsT, rhs, start=True, stop=True)
                    zot = zop.tile([128, ncol], F32, tag="zot")
                    nc.vector.tensor_scalar(out=zot[0:osz, 0:nsz], in0=mps[0:osz, 0:nsz],
                                            scalar1=b3c[0:osz, oi:oi + 1], scalar2=None,
                                            op0=mybir.AluOpType.add)
                    nc.sync.dma_start(out=zo_d[b, o0:o0 + osz, n0:n0 + nsz],
                                      in_=zot[0:osz, 0:nsz])

        # ---------------- final stats ----------------
        rps = sm_ps.tile([pb, 1], F32, tag="sm")
        nc.tensor.matmul(rps[:], ridgeB[:], ones[0:V, :], start=True, stop=True)
        lps = sm_ps.tile([pb, 1], F32, tag="sm")
        nc.tensor.matmul(lps[:], lossB[:], ones[0:c, :], start=True, stop=True)
        stats = smp.tile([pb, 2], F32)
        nc.vector.tensor_copy(stats[:, 0:1], rps[:])
        nc.vector.tensor_copy(stats[:, 1:2], lps[:])
        nc.sync.dma_start(out=st_d[:], in_=stats[:])

    nc.compile()
    return nc


_cache = {}


def _get_program(key):
    if key not in _cache:
        _cache[key] = build_program(conv_mode=key[0], mm3_mode=key[1], band=key[2])
    return _cache[key]


def kernel(x, A, w1, b1, w2, b2, w3, b3):
    x = np.ascontiguousarray(np.asarray(x, dtype=np.float32))
    key = (CONV_MODE, MM3_MODE, BAND)
    nc = _get_program(key)
    base = {
        "w1": np.ascontiguousarray(np.asarray(w1, np.float32)),
        "b1": np.ascontiguousarray(np.asarray(b1, np.float32)),
        "w2": np.ascontiguousarray(np.asarray(w2, np.float32)),
        "b2": np.ascontiguousarray(np.asarray(b2, np.float32)),
        "w3": np.ascontiguousarray(np.asarray(w3, np.float32)),
        "b3": np.ascontiguousarray(np.asarray(b3, np.float32)),
        "A": np.ascontiguousarray(np.asarray(A, np.float32)),
    }
    in_maps = []
    for i in range(NCORES):
        m = dict(base)
        m["x"] = np.ascontiguousarray(x[i * PB:(i + 1) * PB].reshape(PB, C, T * V))
        in_maps.append(m)

    trace = os.environ.get("K_TRACE", "0") == "1"
    res = run_bass_kernel_spmd(nc, in_maps, list(range(NCORES)), trace=trace)
    if trace:
        kernel.last_exec_ns = res.exec_time_ns
        kernel.last_results = res

    zo = np.concatenate([res.results[i]["zo"].reshape(PB, O, T, V) for i in range(NCORES)], axis=0)
    stats = np.stack([res.results[i]["stats"] for i in range(NCORES)])  # [ncores, pb, 2]
    ridge = stats[:, :, 0].reshape(B)
    loss = np.float32(stats[:, :, 1].sum() / (B * V * C * T))
    return zo, loss, ridge


kernel.last_exec_ns = None
